# revision 35
# baseline (speedup 1.0000x reference)
"""Trainium2 Bass kernel for nn_Connect4CNN_Mk5 (dense_cnn).

Strategy (pure data parallel, 8 cores x 4096 samples):
  - batch tile = 128 samples on SBUF partitions.
  - 3x3 SAME convs on the 6x7 board are lowered to "row-pair" dense matmuls:
    per output row y, out_row[y] += T_{dy} @ in_row[y+dy], dy in {-1,0,1},
    where T_dy is a dense 224x224 (32ch x 7cols) matrix built on host from
    the conv weights (x-band made dense; 2.3x FLOP overhead but full PE
    utilization with K=112 half-row chunks, M=128 batch, N=224).
  - Activations live on-chip in two orientations:
      X   [128b, 16g, 6y, 14(c2,x)]  (batch-on-partition; GroupNorm native)
      X_T [112f, 12chunk, 128b]      (feature-on-partition; matmul lhsT)
    with a PE transpose per half-row chunk each layer.
  - GroupNorm stats via tensor_reduce(XY) of x and x^2 (square on ACT),
    rstd = exp(-0.5*ln(var+eps)) to stay inside one ACT table set,
    apply via per-group tensor_scalar (x-mean)*rstd.
  - Heads (LN 128/64) via bn_stats/bn_aggr + fused ACT relu(x*rstd+nb).
  - tanh for the value head via exp: tanh(z) = 2/(1+exp(-2z)) - 1.

Feature order ("trunk order"): f = g*84 + y*14 + c2*7 + x, channel c = 2g+c2.
All host-side weight matrices are permuted to match, including sd_w.
"""

import numpy as np

ROWS, COLS, FILT, GROUPS = 6, 7, 32, 16
EPS = 1e-5
NCORES = 8
B_FULL = 32768
NB = B_FULL // NCORES          # 4096 per core
PT = 128                       # batch tile (partition dim)
NT = NB // PT                  # 32 tiles per core
CELLS = ROWS * COLS            # 42
NF = FILT * CELLS              # 1344
ROWF = FILT * COLS             # 224 features per board row
HALF = ROWF // 2               # 112

_NC_CACHE = {}


# ---------------------------------------------------------------------------
# host-side weight construction
# ---------------------------------------------------------------------------

def _pair_perm():
    """perm224[p] = c*7+x source index for pair-major position p=(c//2)*14+(c%2)*7+x."""
    src = np.zeros(ROWF, np.int64)
    for c in range(FILT):
        for x in range(COLS):
            p = (c // 2) * 14 + (c % 2) * 7 + x
            src[p] = c * COLS + x
    return src


def _row_pair_mats(w):
    """w [O=32, I=32, 3, 3] -> T [3(dy=-1,0,1), 224 in(pair-major), 224 out(pair-major)]."""
    f32 = np.float32
    base = np.zeros((3, FILT, COLS, FILT, COLS), f32)  # [dyi, ci, xi, co, xo]
    for dyi, dy in enumerate((-1, 0, 1)):
        ky = 1 - dy  # y_in - y_out + 1 with dy = y_out - y_in
        for dx in (-1, 0, 1):
            kx = 1 - dx
            for xi in range(COLS):
                xo = xi + dx
                if 0 <= xo < COLS:
                    base[dyi, :, xi, :, xo] = w[:, :, ky, kx].T  # [ci, co]
    T = base.reshape(3, ROWF, ROWF)
    perm = _pair_perm()
    return T[:, perm][:, :, perm]


def _trunk_feature_map():
    """c_of_f[f] for trunk order f = y*224 + g*14 + c2*7 + x (y-major, pair-major cols)."""
    f = np.arange(NF)
    r = f % ROWF
    g = r // 14
    c2 = (r % 14) // 7
    c = 2 * g + c2
    return c, f // ROWF, r % 7


def _host_weights(inp):
    f32 = np.float32
    hw = {}

    # trunk conv blocks: wt [36, 112, 224]
    convs = [inp[f'rb{i}_w{j}'] for i in (1, 2, 3) for j in (1, 2)]
    wt = np.zeros((36, HALF, ROWF), f32)
    for k, w in enumerate(convs):
        T = _row_pair_mats(np.asarray(w, f32))
        for h in (0, 1):
            for dyi in range(3):
                wt[(k * 2 + h) * 3 + dyi] = T[dyi, HALF * h:HALF * (h + 1), :]
    hw['wtrunk'] = wt

    # bank-merged variant: wt2 [12, 112, 1408] per (k, h):
    #   [0:480]    P01  = [T_0  | 0pad | T_+1]   (rows yp, yp+1   for even yp)
    #   [480:960]  PM10 = [T_-1 | 0pad | T_0 ]   (rows yp-1, yp   for odd yp)
    #   [960:1184] TM1  = T_-1                   (single row yp-1)
    #   [1184:]    TP1  = T_+1                   (single row yp+1)
    pad = np.zeros((HALF, 32), f32)
    wt2 = np.zeros((12, HALF, 1408), f32)
    for k in range(6):
        for h in (0, 1):
            tm1 = wt[(k * 2 + h) * 3 + 0]
            t0 = wt[(k * 2 + h) * 3 + 1]
            tp1 = wt[(k * 2 + h) * 3 + 2]
            wt2[k * 2 + h, :, 0:480] = np.concatenate([t0, pad, tp1], axis=1)
            wt2[k * 2 + h, :, 480:960] = np.concatenate([tm1, pad, t0], axis=1)
            wt2[k * 2 + h, :, 960:1184] = tm1
            wt2[k * 2 + h, :, 1184:1408] = tp1
    hw['wt2'] = wt2

    # conv0 dense: m0 [126, 1344], in-feature = ci*42 + yi*7 + xi, out = yo*224 + pair(co,xo)
    w0 = np.asarray(inp['conv0_w'], f32)  # [32, 3, 3, 3]
    m0 = np.zeros((3 * CELLS, ROWS * ROWF), f32)
    for yo in range(ROWS):
        for yi in range(max(0, yo - 1), min(ROWS, yo + 2)):
            ky = yi - yo + 1
            for dx in (-1, 0, 1):
                kx = 1 - dx
                for xi in range(COLS):
                    xo = xi + dx
                    if not (0 <= xo < COLS):
                        continue
                    for co in range(FILT):
                        p = (co // 2) * 14 + (co % 2) * 7 + xo
                        for ci in range(3):
                            m0[ci * CELLS + yi * COLS + xi, yo * ROWF + p] = \
                                w0[co, ci, ky, kx]
    hw['m0'] = m0

    # sd: [12, 112, 128]; chunk k = 2y+h, row j -> (c,x); col m = out feature
    sd_w = np.asarray(inp['sd_w'], f32)  # [128, 1344] cols = c*42+y*7+x
    sdr = np.zeros((12, HALF, 128), f32)
    j = np.arange(HALF)
    gl, c2, x = j // 14, (j // 7) % 2, j % 7
    for y in range(ROWS):
        for h in (0, 1):
            c = 16 * h + 2 * gl + c2
            sdr[2 * y + h] = sd_w[:, c * CELLS + y * COLS + x].T
    hw['sdw'] = sdr

    hw['pw1t'] = np.ascontiguousarray(np.asarray(inp['p_w1'], f32).T)
    hw['pw2t'] = np.ascontiguousarray(np.asarray(inp['p_w2'], f32).T)
    hw['pw3t'] = np.ascontiguousarray(np.asarray(inp['p_w3'], f32).T)  # [128,7]
    hw['vw1t'] = np.ascontiguousarray(np.asarray(inp['v_w1'], f32).T)  # [128,64]
    hw['vw2t'] = np.ascontiguousarray(np.asarray(inp['v_w2'], f32).T)  # [64,64]
    hw['vw3t'] = np.ascontiguousarray(np.asarray(inp['v_w3'], f32).T)  # [64,1]
    hw['pb3'] = np.asarray(inp['p_bias3'], f32)
    hw['vb3'] = np.asarray(inp['v_bias3'], f32)
    return hw


def _gn_gamma_names():
    return [('gn0_g', 'gn0_b')] + [(f'rb{i}_g{j}', f'rb{i}_b{j}')
                                   for i in (1, 2, 3) for j in (1, 2)]


def _ln_gamma_names():
    return [('sd_g', 'sd_b'), ('p_g1', 'p_b1'), ('p_g2', 'p_b2'),
            ('v_g1', 'v_b1'), ('v_g2', 'v_b2')]


def _is_trivial_affine(inp):
    for g, b in _gn_gamma_names() + _ln_gamma_names():
        if not (np.all(np.asarray(inp[g]) == 1.0) and np.all(np.asarray(inp[b]) == 0.0)):
            return False
    return True


def _nontrivial_affine_arrays(inp):
    """Expanded per-feature gamma/beta in trunk order for the 7 GN layers,
    plus raw head LN gammas/betas."""
    f32 = np.float32
    c_of_f, _, _ = _trunk_feature_map()
    out = {}
    for li, (gn, bn) in enumerate(_gn_gamma_names()):
        out[f'gng{li}'] = np.asarray(inp[gn], f32)[c_of_f].copy()
        out[f'gnb{li}'] = np.asarray(inp[bn], f32)[c_of_f].copy()
    for li, (gn, bn) in enumerate(_ln_gamma_names()):
        out[f'lng{li}'] = np.asarray(inp[gn], f32).copy()
        out[f'lnb{li}'] = np.asarray(inp[bn], f32).copy()
    return out


# ---------------------------------------------------------------------------
# pure-numpy emulation of the device algorithm (host validation only)
# ---------------------------------------------------------------------------

def _np_gn(rows, gvec=None, bvec=None):
    """rows [B, 6, 224(pair-major)] -> normalized trunk X [B, 6, 224]."""
    B = rows.shape[0]
    t = rows.reshape(B, 6, 16, 14)
    m = t.mean(axis=(1, 3), keepdims=True)
    v = ((t - m) ** 2).mean(axis=(1, 3), keepdims=True)
    xh = (t - m) * np.exp(-0.5 * np.log(v + EPS))
    if gvec is not None:
        xh = xh * gvec.reshape(1, 6, 16, 14) + bvec.reshape(1, 6, 16, 14)
    return xh.reshape(B, 6, ROWF)


def _np_conv(X, wt, k):
    """X trunk [B, 6, 224] -> rows [B, 6, 224] via row-pair matmuls of conv k (1-based)."""
    B = X.shape[0]
    rows = np.zeros((B, 6, ROWF), np.float32)
    for yp in range(ROWS):
        for h in (0, 1):
            chunk = X[:, yp, HALF * h:HALF * (h + 1)]
            for dyi, dy in enumerate((-1, 0, 1)):
                y = yp + dy
                if 0 <= y < ROWS:
                    rows[:, y, :] += chunk @ wt[((k - 1) * 2 + h) * 3 + dyi]
    return rows


def _np_ln(x, g=None, b=None):
    m = x.mean(-1, keepdims=True)
    v = ((x - m) ** 2).mean(-1, keepdims=True)
    xh = (x - m) * np.exp(-0.5 * np.log(v + EPS))
    if g is not None:
        xh = xh * g + b
    return xh


def _np_forward(inp):
    """Emulate the device computation on host; returns (p, v)."""
    hw = _host_weights(inp)
    trivial = _is_trivial_affine(inp)
    aff = None if trivial else _nontrivial_affine_arrays(inp)

    board = np.asarray(inp['board']).reshape(-1, CELLS)
    bf = board.astype(np.float32)
    sq = bf * bf
    X0 = np.concatenate([(sq + bf) * 0.5, (sq - bf) * 0.5, 1.0 - sq], axis=1)

    def gn_args(li):
        return (None, None) if trivial else (aff[f'gng{li}'], aff[f'gnb{li}'])

    def ln_args(li):
        return (None, None) if trivial else (aff[f'lng{li}'], aff[f'lnb{li}'])

    rows = (X0 @ hw['m0']).reshape(-1, 6, ROWF)
    X = np.maximum(_np_gn(rows, *gn_args(0)), 0.0)
    k = 1
    for blk in range(3):
        T1 = np.maximum(_np_gn(_np_conv(X, hw['wtrunk'], k), *gn_args(k)), 0.0)
        T2 = _np_gn(_np_conv(T1, hw['wtrunk'], k + 1), *gn_args(k + 1))
        X = np.maximum(T2 + X, 0.0)
        k += 2

    B = X.shape[0]
    fpre = np.zeros((B, 128), np.float32)
    for y in range(ROWS):
        for h in (0, 1):
            chunk = X[:, y, HALF * h:HALF * (h + 1)]
            fpre += chunk @ hw['sdw'][2 * y + h]
    f = np.maximum(_np_ln(fpre, *ln_args(0)), 0.0)

    p = np.maximum(_np_ln(f @ hw['pw1t'], *ln_args(1)), 0.0)
    p = np.maximum(_np_ln(p @ hw['pw2t'], *ln_args(2)), 0.0)
    p = p @ hw['pw3t'] + hw['pb3']

    v = np.maximum(_np_ln(f @ hw['vw1t'], *ln_args(3)), 0.0)
    v = np.maximum(_np_ln(v @ hw['vw2t'], *ln_args(4)), 0.0)
    z = v @ hw['vw3t'] + hw['vb3']
    v = np.tanh(z)[:, 0]
    return p, v


# ---------------------------------------------------------------------------
# bass kernel builder
# ---------------------------------------------------------------------------

def _build_bass(trivial, n_tiles=NT, dbg=False):
    import concourse.bass as bass  # noqa: F401
    from concourse import bacc
    import concourse.tile as tile
    import concourse.mybir as mybir
    from concourse.masks import make_identity
    from contextlib import ExitStack

    f32 = mybir.dt.float32
    f32r = mybir.dt.float32r
    i32 = mybir.dt.int32
    AF = mybir.ActivationFunctionType
    OP = mybir.AluOpType
    AX = mybir.AxisListType

    nb = n_tiles * PT
    nc = bacc.Bacc()

    board_d = nc.declare_dram_parameter("board", [nb, CELLS], i32, isOutput=False)
    m0_d = nc.declare_dram_parameter("m0", [3 * CELLS, ROWS * ROWF], f32r, isOutput=False)
    wt_d = nc.declare_dram_parameter("wt2", [12, HALF, 1408], f32r, isOutput=False)
    sd_d = nc.declare_dram_parameter("sdw", [12, HALF, 128], f32r, isOutput=False)
    pw1_d = nc.declare_dram_parameter("pw1t", [128, 128], f32r, isOutput=False)
    pw2_d = nc.declare_dram_parameter("pw2t", [128, 128], f32r, isOutput=False)
    pw3_d = nc.declare_dram_parameter("pw3t", [128, 7], f32, isOutput=False)
    vw1_d = nc.declare_dram_parameter("vw1t", [128, 64], f32r, isOutput=False)
    vw2_d = nc.declare_dram_parameter("vw2t", [64, 64], f32r, isOutput=False)
    vw3_d = nc.declare_dram_parameter("vw3t", [64, 1], f32, isOutput=False)
    pb3_d = nc.declare_dram_parameter("pb3", [7], f32, isOutput=False)
    vb3_d = nc.declare_dram_parameter("vb3", [1], f32, isOutput=False)
    if not trivial:
        gng_d = nc.declare_dram_parameter("gng", [7, NF], f32, isOutput=False)
        gnb_d = nc.declare_dram_parameter("gnb", [7, NF], f32, isOutput=False)
        lng_d = [nc.declare_dram_parameter(f"lng{i}", [d], f32, isOutput=False)
                 for i, d in enumerate((128, 128, 128, 64, 64))]
        lnb_d = [nc.declare_dram_parameter(f"lnb{i}", [d], f32, isOutput=False)
                 for i, d in enumerate((128, 128, 128, 64, 64))]
    p_d = nc.declare_dram_parameter("p_out", [nb, 7], f32, isOutput=True)
    v_d = nc.declare_dram_parameter("v_out", [nb, 1], f32, isOutput=True)
    if dbg:
        dbg_x0_d = nc.declare_dram_parameter("dbg_x0", [128, 126], f32, isOutput=True)
        dbg_rows_d = nc.declare_dram_parameter("dbg_rows", [128, 6, 256], f32, isOutput=True)
        dbg_X_d = nc.declare_dram_parameter("dbg_X", [128, 6, 224], f32, isOutput=True)
        dbg_xt_d = nc.declare_dram_parameter("dbg_xt", [112, 12, 128], f32, isOutput=True)
        dbg_rows1_d = nc.declare_dram_parameter("dbg_rows1", [128, 6, 256], f32, isOutput=True)
        dbg_f_d = nc.declare_dram_parameter("dbg_f", [128, 128], f32, isOutput=True)

    with tile.TileContext(nc) as tc, ExitStack() as ctx:
        const = ctx.enter_context(tc.tile_pool(name="const", bufs=1))
        io = ctx.enter_context(tc.tile_pool(name="io", bufs=3))
        xbuf = ctx.enter_context(tc.tile_pool(name="xbuf", bufs=5))
        xtp = ctx.enter_context(tc.tile_pool(name="xtp", bufs=3))
        scrp = ctx.enter_context(tc.tile_pool(name="scrp", bufs=2))
        smalls = ctx.enter_context(tc.tile_pool(name="smalls", bufs=2))
        heads = ctx.enter_context(tc.tile_pool(name="heads", bufs=2))
        convps = ctx.enter_context(tc.tile_pool(name="convps", bufs=2, space="PSUM"))
        trps = ctx.enter_context(tc.tile_pool(name="trps", bufs=1, space="PSUM"))
        smallps = ctx.enter_context(tc.tile_pool(name="smallps", bufs=1, space="PSUM"))

        # ---- constants
        m0_sb = const.tile([3 * CELLS, ROWS * ROWF], f32r, tag="m0")
        nc.sync.dma_start(out=m0_sb, in_=m0_d[:, :])
        wt_sb = const.tile([HALF, 12, 1408], f32r, tag="wt")
        nc.sync.dma_start(out=wt_sb, in_=wt_d[:, :, :].rearrange("k p n -> p k n"))
        sd_sb = const.tile([HALF, 12, 128], f32r, tag="sd")
        nc.sync.dma_start(out=sd_sb, in_=sd_d[:, :, :].rearrange("k p n -> p k n"))
        pw1_sb = const.tile([128, 128], f32r, tag="pw1")
        nc.sync.dma_start(out=pw1_sb, in_=pw1_d[:, :])
        pw2_sb = const.tile([128, 128], f32r, tag="pw2")
        nc.sync.dma_start(out=pw2_sb, in_=pw2_d[:, :])
        pw3_sb = const.tile([128, 7], f32, tag="pw3")
        nc.sync.dma_start(out=pw3_sb, in_=pw3_d[:, :])
        vw1_sb = const.tile([128, 64], f32r, tag="vw1")
        nc.sync.dma_start(out=vw1_sb, in_=vw1_d[:, :])
        vw2_sb = const.tile([64, 64], f32r, tag="vw2")
        nc.sync.dma_start(out=vw2_sb, in_=vw2_d[:, :])
        vw3_sb = const.tile([64, 1], f32, tag="vw3")
        nc.sync.dma_start(out=vw3_sb, in_=vw3_d[:, :])
        pb3_sb = const.tile([128, 7], f32, tag="pb3")
        nc.sync.dma_start(out=pb3_sb, in_=pb3_d[:].partition_broadcast(128))
        vb3_sb = const.tile([128, 1], f32, tag="vb3")
        nc.sync.dma_start(out=vb3_sb, in_=vb3_d[:].partition_broadcast(128))
        eps_t = const.tile([128, 1], f32, tag="eps")
        nc.vector.memset(eps_t, EPS)
        ident = const.tile([128, 128], f32, tag="ident")
        make_identity(nc, ident)
        if not trivial:
            gng_sb = const.tile([128, 7, NF], f32, tag="gng")
            nc.sync.dma_start(out=gng_sb, in_=gng_d[:, :].partition_broadcast(128))
            gnb_sb = const.tile([128, 7, NF], f32, tag="gnb")
            nc.sync.dma_start(out=gnb_sb, in_=gnb_d[:, :].partition_broadcast(128))
            lng_sb, lnb_sb = [], []
            for i, d in enumerate((128, 128, 128, 64, 64)):
                gt = const.tile([128, d], f32, tag=f"lng{i}")
                nc.sync.dma_start(out=gt, in_=lng_d[i][:].partition_broadcast(128))
                bt = const.tile([128, d], f32, tag=f"lnb{i}")
                nc.sync.dma_start(out=bt, in_=lnb_d[i][:].partition_broadcast(128))
                lng_sb.append(gt)
                lnb_sb.append(bt)

        # ---- helpers -----------------------------------------------------
        def gn_apply(psc, li, relu, resid):
            """psc [128, 6, 256] conv psum -> new trunk X [128, 6, 224]."""
            view = psc[:, :, 0:ROWF].rearrange("p y (g d) -> p g y d", d=14)
            sums = smalls.tile([128, 16], f32, tag="sums")
            nc.vector.tensor_reduce(out=sums, in_=view, axis=AX.XY, op=OP.add)
            scr = scrp.tile([128, 16, 6, 14], f32, tag="scr")
            nc.scalar.activation(out=scr, in_=view, func=AF.Square)
            sumsq = smalls.tile([128, 16], f32, tag="sumsq")
            nc.vector.tensor_reduce(out=sumsq, in_=scr, axis=AX.XY, op=OP.add)
            mean = smalls.tile([128, 16], f32, tag="mean")
            nc.vector.tensor_scalar_mul(mean, sums, 1.0 / 84.0)
            var = smalls.tile([128, 16], f32, tag="var")
            nc.vector.tensor_scalar(var, sumsq, 1.0 / 84.0, None, OP.mult)
            msq = smalls.tile([128, 16], f32, tag="msq")
            nc.vector.tensor_tensor(msq, mean, mean, OP.mult)
            nc.vector.tensor_tensor(var, var, msq, OP.subtract)
            std = smalls.tile([128, 16], f32, tag="std")
            nc.scalar.activation(out=std, in_=var, func=AF.Sqrt, bias=eps_t)
            rstd = smalls.tile([128, 16], f32, tag="rstd")
            nc.vector.reciprocal(out=rstd, in_=std)
            # apply: X = (x - mean_g) * rstd_g via two broadcast tensor_tensor
            X = xbuf.tile([128, ROWS, ROWF], f32, tag="X")
            pview = psc[:, :, 0:ROWF].rearrange("p y (g d) -> p y g d", d=14)
            xview = X.rearrange("p y (g d) -> p y g d", d=14)
            mb = mean[:, None, :, None].to_broadcast((128, ROWS, 16, 14))
            rb = rstd[:, None, :, None].to_broadcast((128, ROWS, 16, 14))
            nc.vector.tensor_tensor(xview, pview, mb, OP.subtract)
            nc.vector.tensor_tensor(xview, xview, rb, OP.mult)
            if not trivial:
                gv = gng_sb[:, li, :].rearrange("p (a b) -> p a b", a=ROWS)
                bv = gnb_sb[:, li, :].rearrange("p (a b) -> p a b", a=ROWS)
                nc.vector.tensor_tensor(X, X, gv, OP.mult)
                nc.vector.tensor_tensor(X, X, bv, OP.add)
            if resid is not None:
                nc.vector.tensor_tensor(X, X, resid, OP.add)
            if relu:
                nc.vector.tensor_scalar_max(X, X, 0.0)
            return X

        def transposes(X, relu_in_copy=False):
            """trunk X -> X_T sbuf tile [112, 12, 128].

            4 chunk-transposes share one psum bank; one DVE copy per bank
            (optionally fused with relu when X itself stays pre-relu)."""
            xt = xtp.tile([HALF, 12, 128], f32r, tag="xt")
            for j in range(3):
                tp = trps.tile([HALF, 4, 128], f32, tag="trp")
                for q in range(4):
                    k = 4 * j + q
                    y, h = k // 2, k % 2
                    nc.tensor.transpose(tp[:, q, :],
                                        X[:, y, HALF * h:HALF * (h + 1)], ident)
                if relu_in_copy:
                    nc.vector.tensor_scalar_max(xt[:, 4 * j:4 * j + 4, :], tp, 0.0)
                else:
                    nc.vector.tensor_copy(out=xt[:, 4 * j:4 * j + 4, :], in_=tp)
            return xt

        def conv(xt, k):
            """X_T chunks -> conv-k psum rows [128, 6, 256].

            start=True clears has_written for the WHOLE psum bank (rows
            share banks in pairs), so only the first matmul issued into
            each bank may carry it; every element's first write then
            overwrites (bit cleared) and the rest accumulate."""
            psc = convps.tile([128, ROWS, 256], f32, tag="convps")
            psf = psc.rearrange("p a b -> p (a b)")
            # segments per (yp): (psum_off, N, wt2_col_off); banks get
            # start on first matmul, stop on last (counted below).
            P01, PM10, TM1, TP1 = 0, 480, 960, 1184
            def segs(yp):
                if yp % 2 == 0:
                    s = [] if yp == 0 else [(256 * (yp - 1), 224, TM1)]
                    return s + [(256 * yp, 480, P01)]
                s = [(256 * (yp - 1), 480, PM10)]
                return s + ([(256 * (yp + 1), 224, TP1)] if yp + 1 < ROWS else [])
            bank_total = {0: 6, 1: 8, 2: 6}
            bank_seen = {0: 0, 1: 0, 2: 0}
            for yp in range(ROWS):
                for h in (0, 1):
                    lhsT = xt[:, 2 * yp + h, :]
                    for off, n, col in segs(yp):
                        b = off // 512
                        start = bank_seen[b] == 0
                        bank_seen[b] += 1
                        stop = bank_seen[b] == bank_total[b]
                        nc.tensor.matmul(
                            psf[:, off:off + n], lhsT,
                            wt_sb[:, (k - 1) * 2 + h, col:col + n],
                            start=start, stop=stop, skip_group_check=True)
            return psc

        def ln_apply(ps, d, li, out_tag):
            """psum [128, d] -> relu(LN(x)) sbuf [128, d]."""
            st = smalls.tile([128, 6], f32, tag="lnst")
            nc.vector.bn_stats(out=st, in_=ps[:, 0:d])
            mv = smalls.tile([128, 2], f32, tag="lnmv")
            nc.vector.bn_aggr(out=mv, in_=st)
            sd_ = smalls.tile([128, 1], f32, tag="lnsd")
            nc.scalar.activation(out=sd_, in_=mv[:, 1:2], func=AF.Sqrt, bias=eps_t)
            rs = smalls.tile([128, 1], f32, tag="lnrs")
            nc.vector.reciprocal(out=rs, in_=sd_)
            nb_ = smalls.tile([128, 1], f32, tag="lnnb")
            nc.vector.tensor_scalar(nb_, mv[:, 0:1], rs, -1.0, OP.mult, OP.mult)
            out = heads.tile([128, d], f32, tag=out_tag)
            if trivial:
                nc.scalar.activation(out=out, in_=ps[:, 0:d], func=AF.Relu,
                                     bias=nb_, scale=rs)
            else:
                nc.scalar.activation(out=out, in_=ps[:, 0:d], func=AF.Identity,
                                     bias=nb_, scale=rs)
                nc.vector.tensor_tensor(out, out, lng_sb[li][:, 0:d], OP.mult)
                nc.vector.tensor_tensor(out, out, lnb_sb[li][:, 0:d], OP.add)
                nc.gpsimd.tensor_scalar_max(out, out, 0.0)
            return out

        def head_transpose(x, d, out_tag, dt_=None):
            """sbuf [128, d] -> sbuf [d, 128]."""
            ps = smallps.tile([d, 128], f32, tag="hps")
            nc.tensor.transpose(ps, x[:, 0:d], ident)
            xt = heads.tile([d, 128], dt_ or f32r, tag=out_tag)
            nc.vector.tensor_copy(out=xt, in_=ps)
            return xt

        # ---- per-batch-tile body ----------------------------------------
        for t in range(n_tiles):
            bs = t * PT
            board_sb = io.tile([128, CELLS], i32, tag="board")
            nc.sync.dma_start(out=board_sb, in_=board_d[bs:bs + PT, :])
            bf = io.tile([128, CELLS], f32, tag="bf")
            nc.vector.tensor_copy(out=bf, in_=board_sb)
            sq = io.tile([128, CELLS], f32, tag="sq")
            nc.vector.tensor_tensor(sq, bf, bf, OP.mult)
            x0 = io.tile([128, 3 * CELLS], f32, tag="x0")
            nc.vector.tensor_tensor(x0[:, 0:CELLS], sq, bf, OP.add)
            nc.vector.tensor_scalar_mul(x0[:, 0:CELLS], x0[:, 0:CELLS], 0.5)
            nc.vector.tensor_tensor(x0[:, CELLS:2 * CELLS], sq, bf, OP.subtract)
            nc.vector.tensor_scalar_mul(x0[:, CELLS:2 * CELLS],
                                        x0[:, CELLS:2 * CELLS], 0.5)
            nc.vector.tensor_scalar(x0[:, 2 * CELLS:3 * CELLS], sq, -1.0, 1.0,
                                    OP.mult, OP.add)
            x0ps = smallps.tile([3 * CELLS, 128], f32, tag="hps")
            nc.tensor.transpose(x0ps, x0, ident)
            x0t = xtp.tile([3 * CELLS, 128], f32r, tag="x0t")
            nc.vector.tensor_copy(out=x0t, in_=x0ps)

            # conv0
            psc = convps.tile([128, ROWS, 256], f32, tag="convps")
            for y in range(ROWS):
                nc.tensor.matmul(psc[:, y, 0:ROWF], x0t,
                                 m0_sb[:, y * ROWF:(y + 1) * ROWF],
                                 start=(y % 2 == 0), stop=(y % 2 == 1),
                                 skip_group_check=True)
            if dbg and t == 0:
                nc.sync.dma_start(out=dbg_x0_d[:, :], in_=x0)
                dcp = heads.tile([128, ROWS, 256], f32, tag="dbgrows")
                nc.vector.tensor_copy(out=dcp, in_=psc)
                nc.sync.dma_start(out=dbg_rows_d[:, :, :], in_=dcp)
            X = gn_apply(psc, 0, relu=True, resid=None)
            xt = transposes(X)
            if dbg and t == 0:
                nc.sync.dma_start(out=dbg_X_d[:, :, :], in_=X)
                nc.sync.dma_start(out=dbg_xt_d[:, :, :], in_=xt)

            k = 1
            for blk in range(3):
                psA = conv(xt, k)
                if dbg and t == 0 and blk == 0:
                    dcp1 = heads.tile([128, ROWS, 256], f32, tag="dbgrows")
                    nc.vector.tensor_copy(out=dcp1, in_=psA)
                    nc.sync.dma_start(out=dbg_rows1_d[:, :, :], in_=dcp1)
                T1 = gn_apply(psA, k, relu=False, resid=None)
                xtA = transposes(T1, relu_in_copy=True)
                psB = conv(xtA, k + 1)
                X = gn_apply(psB, k + 1, relu=True, resid=X)
                xt = transposes(X)
                k += 2

            # sd head
            ps_sd = smallps.tile([128, 128], f32, tag="hps")
            for kk in range(12):
                nc.tensor.matmul(ps_sd, xt[:, kk, :],
                                 sd_sb[:, kk, :],
                                 start=(kk == 0), stop=(kk == 11))
            f = ln_apply(ps_sd, 128, 0, "f")
            if dbg and t == 0:
                nc.sync.dma_start(out=dbg_f_d[:, :], in_=f)
            f_t = head_transpose(f, 128, "f_t")

            # policy head
            ps1 = smallps.tile([128, 128], f32, tag="hps")
            nc.tensor.matmul(ps1, f_t, pw1_sb, start=True, stop=True)
            p1 = ln_apply(ps1, 128, 1, "p1")
            p1t = head_transpose(p1, 128, "p1t")
            ps2 = smallps.tile([128, 128], f32, tag="hps")
            nc.tensor.matmul(ps2, p1t, pw2_sb, start=True, stop=True)
            p2 = ln_apply(ps2, 128, 2, "p2")
            p2t = head_transpose(p2, 128, "p2t", dt_=f32)
            ps3 = smallps.tile([128, 7], f32, tag="hps")
            nc.tensor.matmul(ps3, p2t, pw3_sb, start=True, stop=True)
            pout = io.tile([128, 7], f32, tag="pout")
            nc.vector.tensor_tensor(pout, ps3, pb3_sb, OP.add)
            nc.sync.dma_start(out=p_d[bs:bs + PT, :], in_=pout)

            # value head
            psv1 = smallps.tile([128, 64], f32, tag="hps")
            nc.tensor.matmul(psv1, f_t, vw1_sb, start=True, stop=True)
            v1 = ln_apply(psv1, 64, 3, "v1")
            v1t = head_transpose(v1, 64, "v1t")
            psv2 = smallps.tile([128, 64], f32, tag="hps")
            nc.tensor.matmul(psv2, v1t, vw2_sb, start=True, stop=True)
            v2 = ln_apply(psv2, 64, 4, "v2")
            v2t = head_transpose(v2, 64, "v2t", dt_=f32)
            psv3 = smallps.tile([128, 1], f32, tag="hps")
            nc.tensor.matmul(psv3, v2t, vw3_sb, start=True, stop=True)
            vout = io.tile([128, 1], f32, tag="vout")
            nc.scalar.activation(out=vout, in_=psv3, func=AF.Tanh, bias=vb3_sb)
            nc.sync.dma_start(out=v_d[bs:bs + PT, :], in_=vout)

    nc.finalize()
    return nc


# ---------------------------------------------------------------------------
# entry point
# ---------------------------------------------------------------------------

def kernel(**inputs):
    from concourse.bass_utils import run_bass_kernel_spmd

    inputs = {k: np.asarray(v) for k, v in inputs.items()}
    board = inputs['board'].astype(np.int32).reshape(B_FULL, CELLS)
    hw = _host_weights(inputs)
    trivial = _is_trivial_affine(inputs)

    key = (trivial, NT)
    if key not in _NC_CACHE:
        _NC_CACHE[key] = _build_bass(trivial, NT)
    nc = _NC_CACHE[key]

    base = {k: np.ascontiguousarray(v, dtype=np.float32) for k, v in hw.items()
            if k != 'wtrunk'}
    if not trivial:
        aff = _nontrivial_affine_arrays(inputs)
        base['gng'] = np.stack([aff[f'gng{i}'] for i in range(7)])
        base['gnb'] = np.stack([aff[f'gnb{i}'] for i in range(7)])
        for i in range(5):
            base[f'lng{i}'] = aff[f'lng{i}']
            base[f'lnb{i}'] = aff[f'lnb{i}']

    in_maps = []
    for c in range(NCORES):
        m = dict(base)
        m['board'] = np.ascontiguousarray(board[c * NB:(c + 1) * NB])
        in_maps.append(m)

    res = run_bass_kernel_spmd(nc, in_maps, core_ids=list(range(NCORES)))
    p = np.concatenate([r['p_out'] for r in res.results], axis=0)
    v = np.concatenate([r['v_out'][:, 0] for r in res.results], axis=0)
    return p.astype(np.float32), v.astype(np.float32)


# revision 36
# speedup vs baseline: 1.0740x; 1.0740x over previous
"""Trainium2 Bass kernel for nn_Connect4CNN_Mk5 (dense_cnn).

Strategy (pure data parallel, 8 cores x 4096 samples):
  - batch tile = 128 samples on SBUF partitions.
  - 3x3 SAME convs on the 6x7 board are lowered to "row-pair" dense matmuls:
    per output row y, out_row[y] += T_{dy} @ in_row[y+dy], dy in {-1,0,1},
    where T_dy is a dense 224x224 (32ch x 7cols) matrix built on host from
    the conv weights (x-band made dense; 2.3x FLOP overhead but full PE
    utilization with K=112 half-row chunks, M=128 batch, N=224).
  - Activations live on-chip in two orientations:
      X   [128b, 16g, 6y, 14(c2,x)]  (batch-on-partition; GroupNorm native)
      X_T [112f, 12chunk, 128b]      (feature-on-partition; matmul lhsT)
    with a PE transpose per half-row chunk each layer.
  - GroupNorm stats via tensor_reduce(XY) of x and x^2 (square on ACT),
    rstd = exp(-0.5*ln(var+eps)) to stay inside one ACT table set,
    apply via per-group tensor_scalar (x-mean)*rstd.
  - Heads (LN 128/64) via bn_stats/bn_aggr + fused ACT relu(x*rstd+nb).
  - tanh for the value head via exp: tanh(z) = 2/(1+exp(-2z)) - 1.

Feature order ("trunk order"): f = g*84 + y*14 + c2*7 + x, channel c = 2g+c2.
All host-side weight matrices are permuted to match, including sd_w.
"""

import numpy as np

ROWS, COLS, FILT, GROUPS = 6, 7, 32, 16
EPS = 1e-5
NCORES = 8
B_FULL = 32768
NB = B_FULL // NCORES          # 4096 per core
PT = 128                       # batch tile (partition dim)
NT = NB // PT                  # 32 tiles per core
CELLS = ROWS * COLS            # 42
NF = FILT * CELLS              # 1344
ROWF = FILT * COLS             # 224 features per board row
HALF = ROWF // 2               # 112

_NC_CACHE = {}


# ---------------------------------------------------------------------------
# host-side weight construction
# ---------------------------------------------------------------------------

def _pair_perm():
    """perm224[p] = c*7+x source index for pair-major position p=(c//2)*14+(c%2)*7+x."""
    src = np.zeros(ROWF, np.int64)
    for c in range(FILT):
        for x in range(COLS):
            p = (c // 2) * 14 + (c % 2) * 7 + x
            src[p] = c * COLS + x
    return src


def _row_pair_mats(w):
    """w [O=32, I=32, 3, 3] -> T [3(dy=-1,0,1), 224 in(pair-major), 224 out(pair-major)]."""
    f32 = np.float32
    base = np.zeros((3, FILT, COLS, FILT, COLS), f32)  # [dyi, ci, xi, co, xo]
    for dyi, dy in enumerate((-1, 0, 1)):
        ky = 1 - dy  # y_in - y_out + 1 with dy = y_out - y_in
        for dx in (-1, 0, 1):
            kx = 1 - dx
            for xi in range(COLS):
                xo = xi + dx
                if 0 <= xo < COLS:
                    base[dyi, :, xi, :, xo] = w[:, :, ky, kx].T  # [ci, co]
    T = base.reshape(3, ROWF, ROWF)
    perm = _pair_perm()
    return T[:, perm][:, :, perm]


def _trunk_feature_map():
    """c_of_f[f] for trunk order f = y*224 + g*14 + c2*7 + x (y-major, pair-major cols)."""
    f = np.arange(NF)
    r = f % ROWF
    g = r // 14
    c2 = (r % 14) // 7
    c = 2 * g + c2
    return c, f // ROWF, r % 7


def _host_weights(inp):
    f32 = np.float32
    hw = {}

    # trunk conv blocks: wt [36, 112, 224]
    convs = [inp[f'rb{i}_w{j}'] for i in (1, 2, 3) for j in (1, 2)]
    wt = np.zeros((36, HALF, ROWF), f32)
    for k, w in enumerate(convs):
        T = _row_pair_mats(np.asarray(w, f32))
        for h in (0, 1):
            for dyi in range(3):
                wt[(k * 2 + h) * 3 + dyi] = T[dyi, HALF * h:HALF * (h + 1), :]
    hw['wtrunk'] = wt

    # bank-merged variant: wt2 [12, 112, 1408] per (k, h):
    #   [0:480]    P01  = [T_0  | 0pad | T_+1]   (rows yp, yp+1   for even yp)
    #   [480:960]  PM10 = [T_-1 | 0pad | T_0 ]   (rows yp-1, yp   for odd yp)
    #   [960:1184] TM1  = T_-1                   (single row yp-1)
    #   [1184:]    TP1  = T_+1                   (single row yp+1)
    pad = np.zeros((HALF, 32), f32)
    wt2 = np.zeros((12, HALF, 1408), f32)
    for k in range(6):
        for h in (0, 1):
            tm1 = wt[(k * 2 + h) * 3 + 0]
            t0 = wt[(k * 2 + h) * 3 + 1]
            tp1 = wt[(k * 2 + h) * 3 + 2]
            wt2[k * 2 + h, :, 0:480] = np.concatenate([t0, pad, tp1], axis=1)
            wt2[k * 2 + h, :, 480:960] = np.concatenate([tm1, pad, t0], axis=1)
            wt2[k * 2 + h, :, 960:1184] = tm1
            wt2[k * 2 + h, :, 1184:1408] = tp1
    hw['wt2'] = wt2

    # conv0 dense: m0 [126, 1344], in-feature = ci*42 + yi*7 + xi, out = yo*224 + pair(co,xo)
    w0 = np.asarray(inp['conv0_w'], f32)  # [32, 3, 3, 3]
    m0 = np.zeros((3 * CELLS, ROWS * ROWF), f32)
    for yo in range(ROWS):
        for yi in range(max(0, yo - 1), min(ROWS, yo + 2)):
            ky = yi - yo + 1
            for dx in (-1, 0, 1):
                kx = 1 - dx
                for xi in range(COLS):
                    xo = xi + dx
                    if not (0 <= xo < COLS):
                        continue
                    for co in range(FILT):
                        p = (co // 2) * 14 + (co % 2) * 7 + xo
                        for ci in range(3):
                            m0[ci * CELLS + yi * COLS + xi, yo * ROWF + p] = \
                                w0[co, ci, ky, kx]
    hw['m0'] = m0

    # sd: [12, 112, 128]; chunk k = 2y+h, row j -> (c,x); col m = out feature
    sd_w = np.asarray(inp['sd_w'], f32)  # [128, 1344] cols = c*42+y*7+x
    sdr = np.zeros((12, HALF, 128), f32)
    j = np.arange(HALF)
    gl, c2, x = j // 14, (j // 7) % 2, j % 7
    for y in range(ROWS):
        for h in (0, 1):
            c = 16 * h + 2 * gl + c2
            sdr[2 * y + h] = sd_w[:, c * CELLS + y * COLS + x].T
    hw['sdw'] = sdr

    hw['pw1t'] = np.ascontiguousarray(np.asarray(inp['p_w1'], f32).T)
    hw['pw2t'] = np.ascontiguousarray(np.asarray(inp['p_w2'], f32).T)
    hw['pw3t'] = np.ascontiguousarray(np.asarray(inp['p_w3'], f32).T)  # [128,7]
    hw['vw1t'] = np.ascontiguousarray(np.asarray(inp['v_w1'], f32).T)  # [128,64]
    hw['vw2t'] = np.ascontiguousarray(np.asarray(inp['v_w2'], f32).T)  # [64,64]
    hw['vw3t'] = np.ascontiguousarray(np.asarray(inp['v_w3'], f32).T)  # [64,1]
    hw['pb3'] = np.asarray(inp['p_bias3'], f32)
    hw['vb3'] = np.asarray(inp['v_bias3'], f32)
    return hw


def _gn_gamma_names():
    return [('gn0_g', 'gn0_b')] + [(f'rb{i}_g{j}', f'rb{i}_b{j}')
                                   for i in (1, 2, 3) for j in (1, 2)]


def _ln_gamma_names():
    return [('sd_g', 'sd_b'), ('p_g1', 'p_b1'), ('p_g2', 'p_b2'),
            ('v_g1', 'v_b1'), ('v_g2', 'v_b2')]


def _is_trivial_affine(inp):
    for g, b in _gn_gamma_names() + _ln_gamma_names():
        if not (np.all(np.asarray(inp[g]) == 1.0) and np.all(np.asarray(inp[b]) == 0.0)):
            return False
    return True


def _nontrivial_affine_arrays(inp):
    """Expanded per-feature gamma/beta in trunk order for the 7 GN layers,
    plus raw head LN gammas/betas."""
    f32 = np.float32
    c_of_f, _, _ = _trunk_feature_map()
    out = {}
    for li, (gn, bn) in enumerate(_gn_gamma_names()):
        out[f'gng{li}'] = np.asarray(inp[gn], f32)[c_of_f].copy()
        out[f'gnb{li}'] = np.asarray(inp[bn], f32)[c_of_f].copy()
    for li, (gn, bn) in enumerate(_ln_gamma_names()):
        out[f'lng{li}'] = np.asarray(inp[gn], f32).copy()
        out[f'lnb{li}'] = np.asarray(inp[bn], f32).copy()
    return out


# ---------------------------------------------------------------------------
# pure-numpy emulation of the device algorithm (host validation only)
# ---------------------------------------------------------------------------

def _np_gn(rows, gvec=None, bvec=None):
    """rows [B, 6, 224(pair-major)] -> normalized trunk X [B, 6, 224]."""
    B = rows.shape[0]
    t = rows.reshape(B, 6, 16, 14)
    m = t.mean(axis=(1, 3), keepdims=True)
    v = ((t - m) ** 2).mean(axis=(1, 3), keepdims=True)
    xh = (t - m) * np.exp(-0.5 * np.log(v + EPS))
    if gvec is not None:
        xh = xh * gvec.reshape(1, 6, 16, 14) + bvec.reshape(1, 6, 16, 14)
    return xh.reshape(B, 6, ROWF)


def _np_conv(X, wt, k):
    """X trunk [B, 6, 224] -> rows [B, 6, 224] via row-pair matmuls of conv k (1-based)."""
    B = X.shape[0]
    rows = np.zeros((B, 6, ROWF), np.float32)
    for yp in range(ROWS):
        for h in (0, 1):
            chunk = X[:, yp, HALF * h:HALF * (h + 1)]
            for dyi, dy in enumerate((-1, 0, 1)):
                y = yp + dy
                if 0 <= y < ROWS:
                    rows[:, y, :] += chunk @ wt[((k - 1) * 2 + h) * 3 + dyi]
    return rows


def _np_ln(x, g=None, b=None):
    m = x.mean(-1, keepdims=True)
    v = ((x - m) ** 2).mean(-1, keepdims=True)
    xh = (x - m) * np.exp(-0.5 * np.log(v + EPS))
    if g is not None:
        xh = xh * g + b
    return xh


def _np_forward(inp):
    """Emulate the device computation on host; returns (p, v)."""
    hw = _host_weights(inp)
    trivial = _is_trivial_affine(inp)
    aff = None if trivial else _nontrivial_affine_arrays(inp)

    board = np.asarray(inp['board']).reshape(-1, CELLS)
    bf = board.astype(np.float32)
    sq = bf * bf
    X0 = np.concatenate([(sq + bf) * 0.5, (sq - bf) * 0.5, 1.0 - sq], axis=1)

    def gn_args(li):
        return (None, None) if trivial else (aff[f'gng{li}'], aff[f'gnb{li}'])

    def ln_args(li):
        return (None, None) if trivial else (aff[f'lng{li}'], aff[f'lnb{li}'])

    rows = (X0 @ hw['m0']).reshape(-1, 6, ROWF)
    X = np.maximum(_np_gn(rows, *gn_args(0)), 0.0)
    k = 1
    for blk in range(3):
        T1 = np.maximum(_np_gn(_np_conv(X, hw['wtrunk'], k), *gn_args(k)), 0.0)
        T2 = _np_gn(_np_conv(T1, hw['wtrunk'], k + 1), *gn_args(k + 1))
        X = np.maximum(T2 + X, 0.0)
        k += 2

    B = X.shape[0]
    fpre = np.zeros((B, 128), np.float32)
    for y in range(ROWS):
        for h in (0, 1):
            chunk = X[:, y, HALF * h:HALF * (h + 1)]
            fpre += chunk @ hw['sdw'][2 * y + h]
    f = np.maximum(_np_ln(fpre, *ln_args(0)), 0.0)

    p = np.maximum(_np_ln(f @ hw['pw1t'], *ln_args(1)), 0.0)
    p = np.maximum(_np_ln(p @ hw['pw2t'], *ln_args(2)), 0.0)
    p = p @ hw['pw3t'] + hw['pb3']

    v = np.maximum(_np_ln(f @ hw['vw1t'], *ln_args(3)), 0.0)
    v = np.maximum(_np_ln(v @ hw['vw2t'], *ln_args(4)), 0.0)
    z = v @ hw['vw3t'] + hw['vb3']
    v = np.tanh(z)[:, 0]
    return p, v


# ---------------------------------------------------------------------------
# bass kernel builder
# ---------------------------------------------------------------------------

def _build_bass(trivial, n_tiles=NT, dbg=False):
    import concourse.bass as bass  # noqa: F401
    from concourse import bacc
    import concourse.tile as tile
    import concourse.mybir as mybir
    from concourse.masks import make_identity
    from contextlib import ExitStack

    f32 = mybir.dt.float32
    f32r = mybir.dt.float32r
    i32 = mybir.dt.int32
    AF = mybir.ActivationFunctionType
    OP = mybir.AluOpType
    AX = mybir.AxisListType

    nb = n_tiles * PT
    nc = bacc.Bacc()

    board_d = nc.declare_dram_parameter("board", [nb, CELLS], i32, isOutput=False)
    m0_d = nc.declare_dram_parameter("m0", [3 * CELLS, ROWS * ROWF], f32r, isOutput=False)
    wt_d = nc.declare_dram_parameter("wt2", [12, HALF, 1408], f32r, isOutput=False)
    sd_d = nc.declare_dram_parameter("sdw", [12, HALF, 128], f32r, isOutput=False)
    pw1_d = nc.declare_dram_parameter("pw1t", [128, 128], f32r, isOutput=False)
    pw2_d = nc.declare_dram_parameter("pw2t", [128, 128], f32r, isOutput=False)
    pw3_d = nc.declare_dram_parameter("pw3t", [128, 7], f32, isOutput=False)
    vw1_d = nc.declare_dram_parameter("vw1t", [128, 64], f32r, isOutput=False)
    vw2_d = nc.declare_dram_parameter("vw2t", [64, 64], f32r, isOutput=False)
    vw3_d = nc.declare_dram_parameter("vw3t", [64, 1], f32, isOutput=False)
    pb3_d = nc.declare_dram_parameter("pb3", [7], f32, isOutput=False)
    vb3_d = nc.declare_dram_parameter("vb3", [1], f32, isOutput=False)
    if not trivial:
        gng_d = nc.declare_dram_parameter("gng", [7, NF], f32, isOutput=False)
        gnb_d = nc.declare_dram_parameter("gnb", [7, NF], f32, isOutput=False)
        lng_d = [nc.declare_dram_parameter(f"lng{i}", [d], f32, isOutput=False)
                 for i, d in enumerate((128, 128, 128, 64, 64))]
        lnb_d = [nc.declare_dram_parameter(f"lnb{i}", [d], f32, isOutput=False)
                 for i, d in enumerate((128, 128, 128, 64, 64))]
    p_d = nc.declare_dram_parameter("p_out", [nb, 7], f32, isOutput=True)
    v_d = nc.declare_dram_parameter("v_out", [nb, 1], f32, isOutput=True)
    if dbg:
        dbg_x0_d = nc.declare_dram_parameter("dbg_x0", [128, 126], f32, isOutput=True)
        dbg_rows_d = nc.declare_dram_parameter("dbg_rows", [128, 6, 256], f32, isOutput=True)
        dbg_X_d = nc.declare_dram_parameter("dbg_X", [128, 6, 224], f32, isOutput=True)
        dbg_xt_d = nc.declare_dram_parameter("dbg_xt", [112, 12, 128], f32, isOutput=True)
        dbg_rows1_d = nc.declare_dram_parameter("dbg_rows1", [128, 6, 256], f32, isOutput=True)
        dbg_f_d = nc.declare_dram_parameter("dbg_f", [128, 128], f32, isOutput=True)

    with tile.TileContext(nc) as tc, ExitStack() as ctx:
        const = ctx.enter_context(tc.tile_pool(name="const", bufs=1))
        io = ctx.enter_context(tc.tile_pool(name="io", bufs=3))
        xbuf = ctx.enter_context(tc.tile_pool(name="xbuf", bufs=5))
        xtp = ctx.enter_context(tc.tile_pool(name="xtp", bufs=3))
        scrp = ctx.enter_context(tc.tile_pool(name="scrp", bufs=2))
        smalls = ctx.enter_context(tc.tile_pool(name="smalls", bufs=2))
        heads = ctx.enter_context(tc.tile_pool(name="heads", bufs=2))
        convps = ctx.enter_context(tc.tile_pool(name="convps", bufs=1, space="PSUM"))
        trps = ctx.enter_context(tc.tile_pool(name="trps", bufs=2, space="PSUM"))
        smallps = ctx.enter_context(tc.tile_pool(name="smallps", bufs=2, space="PSUM"))

        # ---- constants
        m0_sb = const.tile([3 * CELLS, ROWS * ROWF], f32r, tag="m0")
        nc.sync.dma_start(out=m0_sb, in_=m0_d[:, :])
        wt_sb = const.tile([HALF, 12, 1408], f32r, tag="wt")
        nc.sync.dma_start(out=wt_sb, in_=wt_d[:, :, :].rearrange("k p n -> p k n"))
        sd_sb = const.tile([HALF, 12, 128], f32r, tag="sd")
        nc.sync.dma_start(out=sd_sb, in_=sd_d[:, :, :].rearrange("k p n -> p k n"))
        pw1_sb = const.tile([128, 128], f32r, tag="pw1")
        nc.sync.dma_start(out=pw1_sb, in_=pw1_d[:, :])
        pw2_sb = const.tile([128, 128], f32r, tag="pw2")
        nc.sync.dma_start(out=pw2_sb, in_=pw2_d[:, :])
        pw3_sb = const.tile([128, 7], f32, tag="pw3")
        nc.sync.dma_start(out=pw3_sb, in_=pw3_d[:, :])
        vw1_sb = const.tile([128, 64], f32r, tag="vw1")
        nc.sync.dma_start(out=vw1_sb, in_=vw1_d[:, :])
        vw2_sb = const.tile([64, 64], f32r, tag="vw2")
        nc.sync.dma_start(out=vw2_sb, in_=vw2_d[:, :])
        vw3_sb = const.tile([64, 1], f32, tag="vw3")
        nc.sync.dma_start(out=vw3_sb, in_=vw3_d[:, :])
        pb3_sb = const.tile([128, 7], f32, tag="pb3")
        nc.sync.dma_start(out=pb3_sb, in_=pb3_d[:].partition_broadcast(128))
        vb3_sb = const.tile([128, 1], f32, tag="vb3")
        nc.sync.dma_start(out=vb3_sb, in_=vb3_d[:].partition_broadcast(128))
        eps_t = const.tile([128, 1], f32, tag="eps")
        nc.vector.memset(eps_t, EPS)
        ident = const.tile([128, 128], f32, tag="ident")
        make_identity(nc, ident)
        if not trivial:
            gng_sb = const.tile([128, 7, NF], f32, tag="gng")
            nc.sync.dma_start(out=gng_sb, in_=gng_d[:, :].partition_broadcast(128))
            gnb_sb = const.tile([128, 7, NF], f32, tag="gnb")
            nc.sync.dma_start(out=gnb_sb, in_=gnb_d[:, :].partition_broadcast(128))
            lng_sb, lnb_sb = [], []
            for i, d in enumerate((128, 128, 128, 64, 64)):
                gt = const.tile([128, d], f32, tag=f"lng{i}")
                nc.sync.dma_start(out=gt, in_=lng_d[i][:].partition_broadcast(128))
                bt = const.tile([128, d], f32, tag=f"lnb{i}")
                nc.sync.dma_start(out=bt, in_=lnb_d[i][:].partition_broadcast(128))
                lng_sb.append(gt)
                lnb_sb.append(bt)

        # ---- helpers -----------------------------------------------------
        def gn_apply(psc, li, relu, resid):
            """psc [128, 6, 256] conv psum -> new trunk X [128, 6, 224]."""
            view = psc[:, :, 0:ROWF].rearrange("p y (g d) -> p g y d", d=14)
            sums = smalls.tile([128, 16], f32, tag="sums")
            nc.vector.tensor_reduce(out=sums, in_=view, axis=AX.XY, op=OP.add)
            scr = scrp.tile([128, 16, 6, 14], f32, tag="scr")
            nc.scalar.activation(out=scr, in_=view, func=AF.Square)
            sumsq = smalls.tile([128, 16], f32, tag="sumsq")
            nc.vector.tensor_reduce(out=sumsq, in_=scr, axis=AX.XY, op=OP.add)
            mean = smalls.tile([128, 16], f32, tag="mean")
            nc.vector.tensor_scalar_mul(mean, sums, 1.0 / 84.0)
            var = smalls.tile([128, 16], f32, tag="var")
            nc.vector.tensor_scalar(var, sumsq, 1.0 / 84.0, None, OP.mult)
            msq = smalls.tile([128, 16], f32, tag="msq")
            nc.vector.tensor_tensor(msq, mean, mean, OP.mult)
            nc.vector.tensor_tensor(var, var, msq, OP.subtract)
            std = smalls.tile([128, 16], f32, tag="std")
            nc.scalar.activation(out=std, in_=var, func=AF.Sqrt, bias=eps_t)
            rstd = smalls.tile([128, 16], f32, tag="rstd")
            nc.vector.reciprocal(out=rstd, in_=std)
            # apply: X = (x - mean_g) * rstd_g via two broadcast tensor_tensor
            X = xbuf.tile([128, ROWS, ROWF], f32, tag="X")
            pview = psc[:, :, 0:ROWF].rearrange("p y (g d) -> p y g d", d=14)
            xview = X.rearrange("p y (g d) -> p y g d", d=14)
            mb = mean[:, None, :, None].to_broadcast((128, ROWS, 16, 14))
            rb = rstd[:, None, :, None].to_broadcast((128, ROWS, 16, 14))
            nc.vector.tensor_tensor(xview, pview, mb, OP.subtract)
            nc.vector.tensor_tensor(xview, xview, rb, OP.mult)
            if not trivial:
                gv = gng_sb[:, li, :].rearrange("p (a b) -> p a b", a=ROWS)
                bv = gnb_sb[:, li, :].rearrange("p (a b) -> p a b", a=ROWS)
                nc.vector.tensor_tensor(X, X, gv, OP.mult)
                nc.vector.tensor_tensor(X, X, bv, OP.add)
            if resid is not None:
                nc.vector.tensor_tensor(X, X, resid, OP.add)
            if relu:
                nc.vector.tensor_scalar_max(X, X, 0.0)
            return X

        def transposes(X, relu_in_copy=False):
            """trunk X -> X_T sbuf tile [112, 12, 128].

            4 chunk-transposes share one psum bank; one DVE copy per bank
            (optionally fused with relu when X itself stays pre-relu)."""
            xt = xtp.tile([HALF, 12, 128], f32r, tag="xt")
            for j in range(3):
                tp = trps.tile([HALF, 4, 128], f32, tag="trp")
                for q in range(4):
                    k = 4 * j + q
                    y, h = k // 2, k % 2
                    nc.tensor.transpose(tp[:, q, :],
                                        X[:, y, HALF * h:HALF * (h + 1)], ident)
                if relu_in_copy:
                    nc.vector.tensor_scalar_max(xt[:, 4 * j:4 * j + 4, :], tp, 0.0)
                else:
                    nc.vector.tensor_copy(out=xt[:, 4 * j:4 * j + 4, :], in_=tp)
            return xt

        def conv(xt, k):
            """X_T chunks -> conv-k psum rows [128, 6, 256].

            start=True clears has_written for the WHOLE psum bank (rows
            share banks in pairs), so only the first matmul issued into
            each bank may carry it; every element's first write then
            overwrites (bit cleared) and the rest accumulate."""
            psc = convps.tile([128, ROWS, 256], f32, tag="convps")
            psf = psc.rearrange("p a b -> p (a b)")
            # segments per (yp): (psum_off, N, wt2_col_off); banks get
            # start on first matmul, stop on last (counted below).
            P01, PM10, TM1, TP1 = 0, 480, 960, 1184
            def segs(yp):
                if yp % 2 == 0:
                    s = [] if yp == 0 else [(256 * (yp - 1), 224, TM1)]
                    return s + [(256 * yp, 480, P01)]
                s = [(256 * (yp - 1), 480, PM10)]
                return s + ([(256 * (yp + 1), 224, TP1)] if yp + 1 < ROWS else [])
            bank_total = {0: 6, 1: 8, 2: 6}
            bank_seen = {0: 0, 1: 0, 2: 0}
            for yp in range(ROWS):
                for h in (0, 1):
                    lhsT = xt[:, 2 * yp + h, :]
                    for off, n, col in segs(yp):
                        b = off // 512
                        start = bank_seen[b] == 0
                        bank_seen[b] += 1
                        stop = bank_seen[b] == bank_total[b]
                        nc.tensor.matmul(
                            psf[:, off:off + n], lhsT,
                            wt_sb[:, (k - 1) * 2 + h, col:col + n],
                            start=start, stop=stop, skip_group_check=True)
            return psc

        def ln_apply(ps, d, li, out_tag):
            """psum [128, d] -> relu(LN(x)) sbuf [128, d]."""
            st = smalls.tile([128, 6], f32, tag="lnst")
            nc.vector.bn_stats(out=st, in_=ps[:, 0:d])
            mv = smalls.tile([128, 2], f32, tag="lnmv")
            nc.vector.bn_aggr(out=mv, in_=st)
            sd_ = smalls.tile([128, 1], f32, tag="lnsd")
            nc.scalar.activation(out=sd_, in_=mv[:, 1:2], func=AF.Sqrt, bias=eps_t)
            rs = smalls.tile([128, 1], f32, tag="lnrs")
            nc.vector.reciprocal(out=rs, in_=sd_)
            nb_ = smalls.tile([128, 1], f32, tag="lnnb")
            nc.vector.tensor_scalar(nb_, mv[:, 0:1], rs, -1.0, OP.mult, OP.mult)
            out = heads.tile([128, d], f32, tag=out_tag)
            if trivial:
                nc.scalar.activation(out=out, in_=ps[:, 0:d], func=AF.Relu,
                                     bias=nb_, scale=rs)
            else:
                nc.scalar.activation(out=out, in_=ps[:, 0:d], func=AF.Identity,
                                     bias=nb_, scale=rs)
                nc.vector.tensor_tensor(out, out, lng_sb[li][:, 0:d], OP.mult)
                nc.vector.tensor_tensor(out, out, lnb_sb[li][:, 0:d], OP.add)
                nc.gpsimd.tensor_scalar_max(out, out, 0.0)
            return out

        def head_transpose(x, d, out_tag, dt_=None):
            """sbuf [128, d] -> sbuf [d, 128]."""
            ps = smallps.tile([d, 128], f32, tag="hps")
            nc.tensor.transpose(ps, x[:, 0:d], ident)
            xt = heads.tile([d, 128], dt_ or f32r, tag=out_tag)
            nc.vector.tensor_copy(out=xt, in_=ps)
            return xt

        # ---- per-batch-tile body ----------------------------------------
        for t in range(n_tiles):
            bs = t * PT
            board_sb = io.tile([128, CELLS], i32, tag="board")
            nc.sync.dma_start(out=board_sb, in_=board_d[bs:bs + PT, :])
            bf = io.tile([128, CELLS], f32, tag="bf")
            nc.vector.tensor_copy(out=bf, in_=board_sb)
            sq = io.tile([128, CELLS], f32, tag="sq")
            nc.vector.tensor_tensor(sq, bf, bf, OP.mult)
            x0 = io.tile([128, 3 * CELLS], f32, tag="x0")
            nc.vector.tensor_tensor(x0[:, 0:CELLS], sq, bf, OP.add)
            nc.vector.tensor_scalar_mul(x0[:, 0:CELLS], x0[:, 0:CELLS], 0.5)
            nc.vector.tensor_tensor(x0[:, CELLS:2 * CELLS], sq, bf, OP.subtract)
            nc.vector.tensor_scalar_mul(x0[:, CELLS:2 * CELLS],
                                        x0[:, CELLS:2 * CELLS], 0.5)
            nc.vector.tensor_scalar(x0[:, 2 * CELLS:3 * CELLS], sq, -1.0, 1.0,
                                    OP.mult, OP.add)
            x0ps = smallps.tile([3 * CELLS, 128], f32, tag="hps")
            nc.tensor.transpose(x0ps, x0, ident)
            x0t = xtp.tile([3 * CELLS, 128], f32r, tag="x0t")
            nc.vector.tensor_copy(out=x0t, in_=x0ps)

            # conv0
            psc = convps.tile([128, ROWS, 256], f32, tag="convps")
            for y in range(ROWS):
                nc.tensor.matmul(psc[:, y, 0:ROWF], x0t,
                                 m0_sb[:, y * ROWF:(y + 1) * ROWF],
                                 start=(y % 2 == 0), stop=(y % 2 == 1),
                                 skip_group_check=True)
            if dbg and t == 0:
                nc.sync.dma_start(out=dbg_x0_d[:, :], in_=x0)
                dcp = heads.tile([128, ROWS, 256], f32, tag="dbgrows")
                nc.vector.tensor_copy(out=dcp, in_=psc)
                nc.sync.dma_start(out=dbg_rows_d[:, :, :], in_=dcp)
            X = gn_apply(psc, 0, relu=True, resid=None)
            xt = transposes(X)
            if dbg and t == 0:
                nc.sync.dma_start(out=dbg_X_d[:, :, :], in_=X)
                nc.sync.dma_start(out=dbg_xt_d[:, :, :], in_=xt)

            k = 1
            for blk in range(3):
                psA = conv(xt, k)
                if dbg and t == 0 and blk == 0:
                    dcp1 = heads.tile([128, ROWS, 256], f32, tag="dbgrows")
                    nc.vector.tensor_copy(out=dcp1, in_=psA)
                    nc.sync.dma_start(out=dbg_rows1_d[:, :, :], in_=dcp1)
                T1 = gn_apply(psA, k, relu=False, resid=None)
                xtA = transposes(T1, relu_in_copy=True)
                psB = conv(xtA, k + 1)
                X = gn_apply(psB, k + 1, relu=True, resid=X)
                xt = transposes(X)
                k += 2

            # sd head
            ps_sd = smallps.tile([128, 128], f32, tag="hps")
            for kk in range(12):
                nc.tensor.matmul(ps_sd, xt[:, kk, :],
                                 sd_sb[:, kk, :],
                                 start=(kk == 0), stop=(kk == 11))
            f = ln_apply(ps_sd, 128, 0, "f")
            if dbg and t == 0:
                nc.sync.dma_start(out=dbg_f_d[:, :], in_=f)
            f_t = head_transpose(f, 128, "f_t")

            # policy head
            ps1 = smallps.tile([128, 128], f32, tag="hps")
            nc.tensor.matmul(ps1, f_t, pw1_sb, start=True, stop=True)
            p1 = ln_apply(ps1, 128, 1, "p1")
            p1t = head_transpose(p1, 128, "p1t")
            ps2 = smallps.tile([128, 128], f32, tag="hps")
            nc.tensor.matmul(ps2, p1t, pw2_sb, start=True, stop=True)
            p2 = ln_apply(ps2, 128, 2, "p2")
            p2t = head_transpose(p2, 128, "p2t", dt_=f32)
            ps3 = smallps.tile([128, 7], f32, tag="hps")
            nc.tensor.matmul(ps3, p2t, pw3_sb, start=True, stop=True)
            pout = io.tile([128, 7], f32, tag="pout")
            nc.vector.tensor_tensor(pout, ps3, pb3_sb, OP.add)
            nc.sync.dma_start(out=p_d[bs:bs + PT, :], in_=pout)

            # value head
            psv1 = smallps.tile([128, 64], f32, tag="hps")
            nc.tensor.matmul(psv1, f_t, vw1_sb, start=True, stop=True)
            v1 = ln_apply(psv1, 64, 3, "v1")
            v1t = head_transpose(v1, 64, "v1t")
            psv2 = smallps.tile([128, 64], f32, tag="hps")
            nc.tensor.matmul(psv2, v1t, vw2_sb, start=True, stop=True)
            v2 = ln_apply(psv2, 64, 4, "v2")
            v2t = head_transpose(v2, 64, "v2t", dt_=f32)
            psv3 = smallps.tile([128, 1], f32, tag="hps")
            nc.tensor.matmul(psv3, v2t, vw3_sb, start=True, stop=True)
            vout = io.tile([128, 1], f32, tag="vout")
            nc.scalar.activation(out=vout, in_=psv3, func=AF.Tanh, bias=vb3_sb)
            nc.sync.dma_start(out=v_d[bs:bs + PT, :], in_=vout)

    nc.finalize()
    return nc


# ---------------------------------------------------------------------------
# entry point
# ---------------------------------------------------------------------------

def kernel(**inputs):
    from concourse.bass_utils import run_bass_kernel_spmd

    inputs = {k: np.asarray(v) for k, v in inputs.items()}
    board = inputs['board'].astype(np.int32).reshape(B_FULL, CELLS)
    hw = _host_weights(inputs)
    trivial = _is_trivial_affine(inputs)

    key = (trivial, NT)
    if key not in _NC_CACHE:
        _NC_CACHE[key] = _build_bass(trivial, NT)
    nc = _NC_CACHE[key]

    base = {k: np.ascontiguousarray(v, dtype=np.float32) for k, v in hw.items()
            if k != 'wtrunk'}
    if not trivial:
        aff = _nontrivial_affine_arrays(inputs)
        base['gng'] = np.stack([aff[f'gng{i}'] for i in range(7)])
        base['gnb'] = np.stack([aff[f'gnb{i}'] for i in range(7)])
        for i in range(5):
            base[f'lng{i}'] = aff[f'lng{i}']
            base[f'lnb{i}'] = aff[f'lnb{i}']

    in_maps = []
    for c in range(NCORES):
        m = dict(base)
        m['board'] = np.ascontiguousarray(board[c * NB:(c + 1) * NB])
        in_maps.append(m)

    res = run_bass_kernel_spmd(nc, in_maps, core_ids=list(range(NCORES)))
    p = np.concatenate([r['p_out'] for r in res.results], axis=0)
    v = np.concatenate([r['v_out'][:, 0] for r in res.results], axis=0)
    return p.astype(np.float32), v.astype(np.float32)
